# revision 41
# baseline (speedup 1.0000x reference)
"""Trainium2 Bass kernel for per-sample channel attention (fp8 DoubleRow).

Reference computation (per sample n of 32):
    e  = x[n].reshape(C, HW)                      # C=512, HW=1024
    q  = sigmoid(relu(e @ wq1) @ wq2)             # [C, HW]
    k  = sigmoid(relu(e @ wk1) @ wk2)             # [C, HW]
    v  = sigmoid(relu(e @ wv1) @ wv2)             # [C, HW]
    s  = q @ k.T / sqrt(C)                        # [C, C]
    o  = softmax(s, axis=-1) @ v                  # [C, HW]

Strategy: data-parallel over batch N across 8 cores (4 samples each),
weights replicated. All big matmuls are fp8 (e4m3) with
perf_mode=DoubleRow; measured back-to-back DR matmuls stream at 216ns
per [128k x 2pair] x [128 x 512] — the FULL 2x-over-bf16 fp8 rate
(~69us/core of pure matmul for the 320 DR matmuls). PSUM accumulates
fp32. Measured HW exec: ~103-107us/core (vs 164us bf16 baseline);
occasional chip-level clock-throttle outliers to ~125us+.

Numerics (absmax rel err ~6.8e-3 vs the 2e-2 gate):
  - k,v are stored as t = tanh(z/2) in (-1,1) (sigmoid = (1+t)/2), q as
    sigma(z) in (0,1). Identities absorb all affine corrections:
      softmax_d(q·sigma_k) == softmax_d(S * 1/(2 sqrt C)),
        S[d,c] = sum_e t_k[d,e]·sigma_q[c,e]   (the q-rowsum term is
        constant per softmax row and cancels)
      P @ v = (P @ t_v + 1)/2                   (softmax rows sum to 1)
    Only two ACT table switches per sample; both prefetched into slack
    via dummy activations.
  - wv2 is quantized with error feedback along the contraction axis so
    each column's sum survives quantization; this kills the dominant
    error term (relu(h) has positive mean, so plain wv2 rounding noise
    is a common mode the softmax averaging cannot cancel). wq2/wk2
    common modes cancel through the softmax and need no treatment.
  - w1,w2 are pre-scaled by 16 on host (fp8 subnormal avoidance); the
    1/256 folds into the ACT scale. Z uses a ones=2.0 vector so the
    reciprocal directly yields 0.5/Z, folding the (1+t)/2 un-mapping
    into the existing per-partition output scale. Output DMA in bf16,
    upcast on host.

Scheduling (the Tile list-scheduler plans via CoreSim):
  - _patch_sched_cost_model doubles the sim's PE cost constants so the
    scheduler sees the TRUE 1.0 cycle/row DoubleRow rate instead of its
    0.5 model; this alone was worth ~12us (without it the sim's 2x-fast
    PE leapfrogs phases around PSUM-slot waits and stalls real HW).
  - software pipeline: Z + the 4 output tiles of sample s-1 run as
    PE-only filler woven between sample s's ACT-fed layer-2 units, so
    exp/reciprocal latency never touches the PE critical path.
  - the 4 softmax-denominator accumulators live in corner columns of
    the previous iteration's two scores PSUM tiles (one per bank; the
    tile-WAR vs the exp reads is the true dependency anyway). That
    frees 2 banks, buying the main [P,2,C] PSUM pool its 4th buffer --
    every pool-rotation wait then lands 4 allocations back, on ACT/DVE
    work that has already drained.
  - relus are split into per-m halves (halves the DVE latency exposed
    to layer-2), layer-1 weight DMAs precede layer-2's, eT prefetch
    rides the sync HWDGE queue mid-iteration, and the epilogue splits
    its tail scale+store into halves alternating DVE/ACT.

Layouts (all SBUF-native, partition-major, pre-swizzled on host):
  eT   [P, KO1, C]  e^T chunks       qT [P, MT_E, C]  sigma_q^T
  kT   [P, MT_E, C] t_k^T            tv [P, MT_C, HW] t_v (natural)
  E    [P, MT_C, C] exp(scores^T)    softmax axis on partitions
DoubleRow contracts k-chunk PAIRS: lhsT [P, 2, M], rhs [P, 2, N=512].
"""

import math

import numpy as np
import ml_dtypes

N, C, H, W, R = 32, 512, 32, 32, 4
HW = H * W            # 1024
HID = HW // R         # 256
NCORES = 8
PER = N // NCORES     # samples per core
P = 128               # partitions

KO1 = HW // P         # 8  k-chunks for layer1
KP1 = KO1 // 2        # 4  DoubleRow pairs for layer1
KO2 = HID // P        # 2  k-chunks for layer2 (one DoubleRow pair)
MT_H = HID // P       # 2  m-tiles of h^T
MT_E = HW // P        # 8  m-tiles of q^T/k^T
KPE = MT_E // 2       # 4  DoubleRow pairs for scores contraction
MT_C = C // P         # 4  m-tiles over C
KPC = MT_C // 2       # 2  DoubleRow pairs for o contraction
NH = HW // C          # 2  halves of HW free dim (512 each)

S1 = 16.0             # host pre-scale on w1 (fp8 range use)
S2 = 16.0             # host pre-scale on w2

_STATE = {}

FP8 = ml_dtypes.float8_e4m3


def _patch_sched_cost_model():
    """Make the Tile scheduler's CoreSim cost model match measured HW.

    CoreSim costs fp8 DoubleRow matmuls at 0.5 cycles/row (107ns @ N=512)
    but the hardware streams them at 1.0 cycle/row (measured 216ns
    back-to-back). The list scheduler therefore simulates a PE twice as
    fast as reality and leapfrogs whole phases around PSUM-slot waits,
    producing avoidable PE stalls on hardware. Doubling the PE cost
    constants (loaded lazily by the Rust cost model from this module)
    restores the true PE:ACT:DVE timing ratios. Sim-only: NEFF codegen
    never reads these.
    """
    import concourse.hw_specs as hs

    spec = hs.TRN2Spec
    if getattr(spec, "_dr_cost_patched", False):
        return
    spec._dr_cost_patched = True
    spec.PE_CYCLE *= 2.0
    spec.PE_CYCLE_PSTATE_MID *= 2.0
    spec.PE_CYCLE_PSTATE_LOW *= 2.0
    spec.PE_SBUF_ACCESS_LATENCY_NS *= 2.0


def _build_nc():
    import concourse.bass as bass  # noqa: F401
    import concourse.mybir as mybir
    import concourse.tile as tile
    from concourse import bacc

    _patch_sched_cost_model()

    f8 = mybir.dt.float8e4
    f32 = mybir.dt.float32
    A = mybir.ActivationFunctionType
    DR = mybir.MatmulPerfMode.DoubleRow
    ALU = mybir.AluOpType

    nc = bacc.Bacc("TRN2")

    xt = nc.dram_tensor("xt", [PER, P, KO1, C], f8, kind="ExternalInput")
    ws1 = {
        kind: nc.dram_tensor(f"w{kind}1", [P, KO1, HID], f8, kind="ExternalInput")
        for kind in "qkv"
    }
    ws2 = {
        kind: nc.dram_tensor(f"w{kind}2", [P, KO2, HW], f8, kind="ExternalInput")
        for kind in "qkv"
    }
    bf16 = mybir.dt.bfloat16
    # output in bf16 (upcast on host): halves output DMA traffic; adds
    # <=2e-3 abs error on ~0.9-scale outputs, well within the error budget
    out = nc.dram_tensor("o", [PER, C, HW], bf16, kind="ExternalOutput")

    # exp scale: logits_eff = S * 1/(2 sqrt C)  (see module docstring)
    exp_scale = 1.0 / (2.0 * math.sqrt(C))
    sig_scale = 1.0 / (S1 * S2)        # sigma(z2) from psum z2' = S1*S2*z2
    tanh_scale = 1.0 / (2.0 * S1 * S2)  # tanh(z2/2)

    with tile.TileContext(nc) as tc:
        with (
            tc.tile_pool(name="singles", bufs=1) as singles,
            tc.tile_pool(name="acts", bufs=2) as acts,
            tc.tile_pool(name="hts", bufs=3) as hts,
            tc.tile_pool(name="obuf", bufs=3) as obuf,
            tc.tile_pool(name="psum", bufs=4, space="PSUM") as psum,
        ):
            w1_sb = {}
            w2_sb = {}
            for kind in "qkv":
                w1_sb[kind] = singles.tile(
                    [P, KO1, HID], f8, tag=f"w1{kind}", name=f"w1{kind}"
                )
                w2_sb[kind] = singles.tile(
                    [P, KO2, HW], f8, tag=f"w2{kind}", name=f"w2{kind}"
                )

            warm_sb = singles.tile([P, C], f8, tag="warm", name="warm")
            nc.gpsimd.memset(warm_sb, 0.0)

            # Input DMAs in need-order: wq1 + eT(s0) first (first matmuls),
            # the rest behind. HWDGE (sync) and SWDGE (gpsimd) run in
            # parallel; split the critical first wave across both.
            nc.sync.dma_start(out=w1_sb["q"], in_=ws1["q"][:])
            eT0 = acts.tile([P, KO1, C], f8, tag="eT", name="eT")
            nc.gpsimd.dma_start(out=eT0[:, 6:, :], in_=xt[0][:, 6:, :])
            nc.sync.dma_start(out=eT0[:, :2, :], in_=xt[0][:, :2, :])
            nc.sync.dma_start(out=eT0[:, 2:4, :], in_=xt[0][:, 2:4, :])
            nc.sync.dma_start(out=eT0[:, 4:6, :], in_=xt[0][:, 4:6, :])
            # layer-1 weights before the layer-2 ones: they are needed
            # first (sample 0's L1k/L1v would stall behind w2 transfers)
            nc.sync.dma_start(out=w1_sb["k"], in_=ws1["k"][:])
            nc.sync.dma_start(out=w1_sb["v"], in_=ws1["v"][:])
            nc.sync.dma_start(out=w2_sb["q"], in_=ws2["q"][:])
            nc.sync.dma_start(out=w2_sb["k"], in_=ws2["k"][:])
            nc.sync.dma_start(out=w2_sb["v"], in_=ws2["v"][:])

            # ones = 2.0 so the Z reciprocal yields 0.5/Z directly
            ones_sb = singles.tile([P, 1], f8, tag="ones", name="ones")
            nc.vector.memset(ones_sb, 2.0)
            actw = singles.tile([P, 1], f32, tag="actw", name="actw")

            # PE clock warm-up (HAM un-throttles after ~3.4us of activity)
            warm_ps = psum.tile([P, 2, C], f32, tag="ps", name="ps")
            for _ in range(7):
                nc.tensor.matmul(
                    warm_ps[:, 0, :], warm_sb[:, :P], warm_sb, start=True, stop=True
                )
            # Preload the sigmoid table set (contains sigmoid+tanh) while
            # DMAs land.
            nc.scalar.activation(actw, warm_sb[:, :1], A.Sigmoid)

            def layer1(kind, eT, kporder):
                # layer 1: h^T[r, c], DoubleRow over k-chunk pairs, then
                # relu PSUM -> fp8 SBUF (both m-tiles in one DVE op). The
                # relu latency is hidden under the NEXT block's matmuls.
                ps1 = psum.tile([P, MT_H, C], f32, tag="ps", name="ps")
                for m in range(MT_H):
                    for kpi, kp in enumerate(kporder):
                        nc.tensor.matmul(
                            ps1[:, m, :],
                            w1_sb[kind][:, 2 * kp : 2 * kp + 2, m * P : (m + 1) * P],
                            eT[:, 2 * kp : 2 * kp + 2, :],
                            start=(kpi == 0),
                            stop=(kpi == KP1 - 1),
                            perf_mode=DR,
                        )
                # relu in two per-m halves: halves the DVE latency exposed
                # to the PE consumer and smooths the DVE queue
                hT = hts.tile([P, KO2, C], f8, tag="hT", name="hT")
                for m in range(MT_H):
                    nc.vector.tensor_scalar_max(hT[:, m, :], ps1[:, m, :], 0.0)
                return hT

            def layer2_qk_unit(kind, hT, dst, mp):
                # one m-pair of layer 2 transposed: z2^T[e, c]
                fn = A.Sigmoid if kind == "q" else A.Tanh
                sc = sig_scale if kind == "q" else tanh_scale
                ps2 = psum.tile([P, 2, C], f32, tag="ps", name="ps")
                for mi in range(2):
                    m = 2 * mp + mi
                    nc.tensor.matmul(
                        ps2[:, mi, :],
                        w2_sb[kind][:, 0:2, m * P : (m + 1) * P],
                        hT[:, 0:2, :],
                        start=True,
                        stop=True,
                        perf_mode=DR,
                    )
                nc.scalar.activation(
                    dst[:, 2 * mp : 2 * mp + 2, :], ps2[:], fn, scale=sc
                )

            def layer2_v_unit(hT, tv, m):
                # one m-tile of v natural: z2[d, e] = (h^T)^T @ w2
                ps2 = psum.tile([P, 2, C], f32, tag="ps", name="ps")
                for nh in range(NH):
                    nc.tensor.matmul(
                        ps2[:, nh, :],
                        hT[:, 0:2, m * P : (m + 1) * P],
                        w2_sb["v"][:, 0:2, nh * C : (nh + 1) * C],
                        start=True,
                        stop=True,
                        perf_mode=DR,
                    )
                nc.scalar.activation(tv[:, m, :], ps2[:], A.Tanh, scale=tanh_scale)

            def scores_unit(qT, kT, E, mp, split_exp=False):
                # one m-pair of S^T[d, c] = sum_e t_k[d,e] sigma_q[c,e];
                # returns the psum tile (its corner hosts a Z accumulator)
                ps = psum.tile([P, 2, C], f32, tag="ps", name="ps")
                for mi in range(2):
                    m = 2 * mp + mi
                    for kp in range(KPE):
                        nc.tensor.matmul(
                            ps[:, mi, :],
                            kT[:, 2 * kp : 2 * kp + 2, m * P : (m + 1) * P],
                            qT[:, 2 * kp : 2 * kp + 2, :],
                            start=(kp == 0),
                            stop=(kp == KPE - 1),
                            perf_mode=DR,
                        )
                if split_exp:
                    # final iteration: per-half exps let the epilogue's Z
                    # start one exp earlier
                    for mi in range(2):
                        nc.scalar.activation(
                            E[:, 2 * mp + mi, :], ps[:, mi, :], A.Exp,
                            scale=exp_scale,
                        )
                else:
                    nc.scalar.activation(
                        E[:, 2 * mp : 2 * mp + 2, :], ps[:, :, :], A.Exp,
                        scale=exp_scale,
                    )
                return ps

            def z_phase(E, scps):
                # softmax denominator rz[c] = 0.5/Z[c] (ones = 2.0). The 4
                # accumulators live in the corner columns of the PREVIOUS
                # iteration's two scores tiles — one per PSUM bank (groups
                # sharing a bank would serialize), and the tile-WAR against
                # the exp reads is exactly the true dependency. This frees
                # two banks, buying the main pool its 4th buffer.
                rz = obuf.tile([P, MT_C], f32, tag="rz", name="rz")
                for m in range(MT_C):
                    pz = scps[m // 2][:, m % 2, C - 1 : C]
                    for k in range(MT_C):
                        nc.tensor.matmul(
                            pz,
                            E[:, k, m * P : (m + 1) * P],
                            ones_sb,
                            start=(k == 0),
                            stop=(k == MT_C - 1),
                        )
                    nc.vector.reciprocal(rz[:, m : m + 1], pz)
                return rz

            def o_unit(s, E, tv, rz, m, split):
                # one m-tile of o[c, e] = (E^T @ t_v) * rz[c] + 0.5
                ob = obuf.tile([P, HW], bf16, tag="ob", name="ob")
                ps = psum.tile([P, 2, C], f32, tag="ps", name="ps")
                out_r = out[s].rearrange("(mo p) e -> p mo e", p=P)
                for kp in range(KPC):
                    for nh in range(NH):
                        nc.tensor.matmul(
                            ps[:, nh, :],
                            E[:, 2 * kp : 2 * kp + 2, m * P : (m + 1) * P],
                            tv[:, 2 * kp : 2 * kp + 2, nh * C : (nh + 1) * C],
                            start=(kp == 0),
                            stop=(kp == KPC - 1),
                            perf_mode=DR,
                        )
                if not split:
                    nc.vector.tensor_scalar(
                        ob[:], ps[:], rz[:, m : m + 1], 0.5, ALU.mult, ALU.add
                    )
                    eng = (nc.sync, nc.scalar)[m % 2]
                    eng.dma_start(out=out_r[:, m, :], in_=ob[:])
                else:
                    # final sample: the epilogue has no matmuls to hide
                    # behind, so the scale+store drain chain is the tail.
                    # Split into halves and alternate the scale between the
                    # DVE and the (now idle) ACT engine — Copy activation
                    # computes ps*rz + 0.5 directly.
                    for nh in range(NH):
                        dst = ob[:, nh * C : (nh + 1) * C]
                        src = ps[:, nh, :]
                        if (m * NH + nh) % 2 == 0:
                            nc.vector.tensor_scalar(
                                dst, src, rz[:, m : m + 1], 0.5, ALU.mult, ALU.add
                            )
                        else:
                            # ACT is idle in the epilogue; Copy computes
                            # ps*rz + 0.5 directly, halving the drain chain
                            nc.scalar.activation(
                                dst, src, A.Copy, bias=0.5, scale=rz[:, m : m + 1]
                            )
                        eng = (nc.sync, nc.scalar)[(m * NH + nh) % 2]
                        eng.dma_start(
                            out=out_r[:, m, nh * C : (nh + 1) * C],
                            in_=dst,
                        )

            # Software pipeline. Per iteration, PE-only units (layer-1
            # blocks, Z, and the PREVIOUS sample's o-tiles) are woven
            # between the ACT-fed layer-2 / scores units so the scalar
            # engine (the second-busiest) never backs the PSUM pool up
            # into the PE. exp/reciprocal of sample s complete during
            # iteration s+1 long before their consumers.
            prev = None
            eT_next = None
            for s in range(PER):
                eT = eT0 if s == 0 else eT_next

                qT = acts.tile([P, MT_E, C], f8, tag="qT", name="qT")
                kT = acts.tile([P, MT_E, C], f8, tag="kT", name="kT")
                tv = acts.tile([P, MT_C, HW], f8, tag="tv", name="tv")

                # sample 0's eT pairs arrive sync(0,1),(2,3),(4,5) +
                # gpsimd(6,7); accumulate in arrival order (order is free)
                kporder = (0, 1, 3, 2) if s == 0 else tuple(range(KP1))
                hq = layer1("q", eT, kporder)
                hk = layer1("k", eT, kporder)
                if prev is not None:
                    # Z early: its reciprocals clear the DVE queue before
                    # the relu halves and o-scales pile up
                    ps_, E_, tv_, scps_ = prev
                    rz_ = z_phase(E_, scps_)
                layer2_qk_unit("q", hq, qT, 0)
                layer2_qk_unit("q", hq, qT, 1)
                hv = layer1("v", eT, kporder)
                if s + 1 < PER:
                    # prefetch next sample's eT on the sync HWDGE queue;
                    # issued after this sample's layer-1 so the transfer
                    # never contends with the startup input DMAs
                    eT_next = acts.tile([P, KO1, C], f8, tag="eT", name="eT")
                    nc.sync.dma_start(out=eT_next, in_=xt[s + 1])
                layer2_qk_unit("q", hq, qT, 2)
                layer2_qk_unit("q", hq, qT, 3)
                layer2_qk_unit("k", hk, kT, 0)
                layer2_qk_unit("k", hk, kT, 1)
                if prev is not None:
                    o_unit(ps_, E_, tv_, rz_, 0, False)
                layer2_qk_unit("k", hk, kT, 2)
                layer2_qk_unit("k", hk, kT, 3)
                if prev is not None:
                    o_unit(ps_, E_, tv_, rz_, 1, False)
                    o_unit(ps_, E_, tv_, rz_, 2, False)
                layer2_v_unit(hv, tv, 0)
                layer2_v_unit(hv, tv, 1)
                layer2_v_unit(hv, tv, 2)
                layer2_v_unit(hv, tv, 3)
                if prev is not None:
                    o_unit(ps_, E_, tv_, rz_, 3, False)
                # prefetch the exp table (depends on the last tanh output)
                nc.scalar.activation(actw, tv[:, MT_C - 1, HW - 1 :], A.Exp)

                E = acts.tile([P, MT_C, C], f8, tag="E", name="E")
                last = s == PER - 1
                scp0 = scores_unit(qT, kT, E, 0, split_exp=last)
                scp1 = scores_unit(qT, kT, E, 1, split_exp=last)
                if not last:
                    # pull the sigmoid-table reload into the next window
                    nc.scalar.activation(actw, E[:, MT_C - 1, :1], A.Sigmoid)
                prev = (s, E, tv, (scp0, scp1))

            # epilogue: final sample's Z + output drain
            s_, E_, tv_, scps_ = prev
            rz_ = z_phase(E_, scps_)
            for m in range(MT_C):
                o_unit(s_, E_, tv_, rz_, m, m == MT_C - 1)

    nc.finalize()
    return nc


def _get_nc():
    if "nc" not in _STATE:
        _STATE["nc"] = _build_nc()
    return _STATE["nc"]


def _quant_ef(a):
    """fp8 quantization with error feedback along axis 0 (contraction axis):
    carry the rounding residual so each column's sum is preserved."""
    out = np.empty(a.shape, dtype=FP8)
    c = np.zeros(a.shape[1:], dtype=np.float32)
    for h in range(a.shape[0]):
        u = a[h] + c
        q = u.astype(FP8)
        c = u - q.astype(np.float32)
        out[h] = q
    return out


def kernel(**inputs):
    x = np.asarray(inputs["x"])

    # host-side reformat to SBUF-native layouts (+ fp8 cast):
    #   x:  [N, C, H, W] -> e^T [N, HW, C] -> [N, P, KO1, C]
    #   w1: [HW, HID] * 16 -> [P, KO1, HID]
    #   w2: [HID, HW] * 16 -> [P, KO2, HW]  (wv2 with error feedback)
    xt = np.ascontiguousarray(
        x.reshape(N, C, HW)
        .transpose(0, 2, 1)
        .reshape(N, KO1, P, C)
        .transpose(0, 2, 1, 3)
    ).astype(FP8)
    w = {}
    for name in ("wq1", "wk1", "wv1"):
        a = (np.asarray(inputs[name]) * S1).astype(FP8)
        w[name] = np.ascontiguousarray(a.reshape(KO1, P, HID).transpose(1, 0, 2))
    for name in ("wq2", "wk2", "wv2"):
        a = np.asarray(inputs[name]).astype(np.float32) * S2
        a8 = _quant_ef(a) if name == "wv2" else a.astype(FP8)
        w[name] = np.ascontiguousarray(a8.reshape(KO2, P, HW).transpose(1, 0, 2))

    nc = _get_nc()

    in_maps = []
    for c in range(NCORES):
        m = {"xt": np.ascontiguousarray(xt[c * PER : (c + 1) * PER])}
        for kind in "qkv":
            m[f"w{kind}1"] = w[f"w{kind}1"]
            m[f"w{kind}2"] = w[f"w{kind}2"]
        in_maps.append(m)

    from concourse.bass_utils import run_bass_kernel_spmd

    res = run_bass_kernel_spmd(
        nc,
        in_maps,
        core_ids=list(range(NCORES)),
        trace=_STATE.get("trace", False),
        **_STATE.get("run_kwargs", {}),
    )
    _STATE["last_result"] = res

    o = np.concatenate([r["o"] for r in res.results], axis=0)
    return o.reshape(N, C, H, W).astype(np.float32)


# revision 43
# speedup vs baseline: 1.0036x; 1.0036x over previous
"""Trainium2 Bass kernel for per-sample channel attention (fp8 DoubleRow).

Reference computation (per sample n of 32):
    e  = x[n].reshape(C, HW)                      # C=512, HW=1024
    q  = sigmoid(relu(e @ wq1) @ wq2)             # [C, HW]
    k  = sigmoid(relu(e @ wk1) @ wk2)             # [C, HW]
    v  = sigmoid(relu(e @ wv1) @ wv2)             # [C, HW]
    s  = q @ k.T / sqrt(C)                        # [C, C]
    o  = softmax(s, axis=-1) @ v                  # [C, HW]

Strategy: data-parallel over batch N across 8 cores (4 samples each),
weights replicated. All big matmuls are fp8 (e4m3) with
perf_mode=DoubleRow; measured back-to-back DR matmuls stream at 216ns
per [128k x 2pair] x [128 x 512] — the FULL 2x-over-bf16 fp8 rate
(~69us/core of pure matmul for the 320 DR matmuls). PSUM accumulates
fp32. Measured HW exec: ~103-107us/core (vs 164us bf16 baseline);
occasional chip-level clock-throttle outliers to ~125us+.

Numerics (absmax rel err ~6.8e-3 vs the 2e-2 gate):
  - k,v are stored as t = tanh(z/2) in (-1,1) (sigmoid = (1+t)/2), q as
    sigma(z) in (0,1). Identities absorb all affine corrections:
      softmax_d(q·sigma_k) == softmax_d(S * 1/(2 sqrt C)),
        S[d,c] = sum_e t_k[d,e]·sigma_q[c,e]   (the q-rowsum term is
        constant per softmax row and cancels)
      P @ v = (P @ t_v + 1)/2                   (softmax rows sum to 1)
    Only two ACT table switches per sample; both prefetched into slack
    via dummy activations.
  - wv2 is quantized with error feedback along the contraction axis so
    each column's sum survives quantization; this kills the dominant
    error term (relu(h) has positive mean, so plain wv2 rounding noise
    is a common mode the softmax averaging cannot cancel). wq2/wk2
    common modes cancel through the softmax and need no treatment.
  - w1,w2 are pre-scaled by 16 on host (fp8 subnormal avoidance); the
    1/256 folds into the ACT scale. Z uses a ones=2.0 vector so the
    reciprocal directly yields 0.5/Z, folding the (1+t)/2 un-mapping
    into the existing per-partition output scale. Output DMA in bf16,
    upcast on host.

Scheduling (the Tile list-scheduler plans via CoreSim):
  - _patch_sched_cost_model doubles the sim's PE cost constants so the
    scheduler sees the TRUE 1.0 cycle/row DoubleRow rate instead of its
    0.5 model; this alone was worth ~12us (without it the sim's 2x-fast
    PE leapfrogs phases around PSUM-slot waits and stalls real HW).
  - software pipeline: Z + the 4 output tiles of sample s-1 run as
    PE-only filler woven between sample s's ACT-fed layer-2 units, so
    exp/reciprocal latency never touches the PE critical path.
  - the 4 softmax-denominator accumulators live in corner columns of
    the previous iteration's two scores PSUM tiles (one per bank; the
    tile-WAR vs the exp reads is the true dependency anyway). That
    frees 2 banks, buying the main [P,2,C] PSUM pool its 4th buffer --
    every pool-rotation wait then lands 4 allocations back, on ACT/DVE
    work that has already drained.
  - relus are split into per-m halves (halves the DVE latency exposed
    to layer-2), layer-1 weight DMAs precede layer-2's, eT prefetch
    rides the sync HWDGE queue mid-iteration, and the epilogue splits
    its tail scale+store into halves alternating DVE/ACT.

Layouts (all SBUF-native, partition-major, pre-swizzled on host):
  eT   [P, KO1, C]  e^T chunks       qT [P, MT_E, C]  sigma_q^T
  kT   [P, MT_E, C] t_k^T            tv [P, MT_C, HW] t_v (natural)
  E    [P, MT_C, C] exp(scores^T)    softmax axis on partitions
DoubleRow contracts k-chunk PAIRS: lhsT [P, 2, M], rhs [P, 2, N=512].
"""

import math

import numpy as np
import ml_dtypes

N, C, H, W, R = 32, 512, 32, 32, 4
HW = H * W            # 1024
HID = HW // R         # 256
NCORES = 8
PER = N // NCORES     # samples per core
P = 128               # partitions

KO1 = HW // P         # 8  k-chunks for layer1
KP1 = KO1 // 2        # 4  DoubleRow pairs for layer1
KO2 = HID // P        # 2  k-chunks for layer2 (one DoubleRow pair)
MT_H = HID // P       # 2  m-tiles of h^T
MT_E = HW // P        # 8  m-tiles of q^T/k^T
KPE = MT_E // 2       # 4  DoubleRow pairs for scores contraction
MT_C = C // P         # 4  m-tiles over C
KPC = MT_C // 2       # 2  DoubleRow pairs for o contraction
NH = HW // C          # 2  halves of HW free dim (512 each)

S1 = 16.0             # host pre-scale on w1 (fp8 range use)
S2 = 16.0             # host pre-scale on w2

_STATE = {}

FP8 = ml_dtypes.float8_e4m3


def _patch_sched_cost_model():
    """Make the Tile scheduler's CoreSim cost model match measured HW.

    CoreSim costs fp8 DoubleRow matmuls at 0.5 cycles/row (107ns @ N=512)
    but the hardware streams them at 1.0 cycle/row (measured 216ns
    back-to-back). The list scheduler therefore simulates a PE twice as
    fast as reality and leapfrogs whole phases around PSUM-slot waits,
    producing avoidable PE stalls on hardware. Doubling the PE cost
    constants (loaded lazily by the Rust cost model from this module)
    restores the true PE:ACT:DVE timing ratios. Sim-only: NEFF codegen
    never reads these.
    """
    import concourse.hw_specs as hs

    spec = hs.TRN2Spec
    if getattr(spec, "_dr_cost_patched", False):
        return
    spec._dr_cost_patched = True
    spec.PE_CYCLE *= 2.0
    spec.PE_CYCLE_PSTATE_MID *= 2.0
    spec.PE_CYCLE_PSTATE_LOW *= 2.0
    spec.PE_SBUF_ACCESS_LATENCY_NS *= 2.0


def _build_nc():
    import concourse.bass as bass  # noqa: F401
    import concourse.mybir as mybir
    import concourse.tile as tile
    from concourse import bacc

    _patch_sched_cost_model()

    f8 = mybir.dt.float8e4
    f32 = mybir.dt.float32
    A = mybir.ActivationFunctionType
    DR = mybir.MatmulPerfMode.DoubleRow
    ALU = mybir.AluOpType

    nc = bacc.Bacc("TRN2")

    xt = nc.dram_tensor("xt", [PER, P, KO1, C], f8, kind="ExternalInput")
    ws1 = {
        kind: nc.dram_tensor(f"w{kind}1", [P, KO1, HID], f8, kind="ExternalInput")
        for kind in "qkv"
    }
    ws2 = {
        kind: nc.dram_tensor(f"w{kind}2", [P, KO2, HW], f8, kind="ExternalInput")
        for kind in "qkv"
    }
    bf16 = mybir.dt.bfloat16
    # output in bf16 (upcast on host): halves output DMA traffic; adds
    # <=2e-3 abs error on ~0.9-scale outputs, well within the error budget
    out = nc.dram_tensor("o", [PER, C, HW], bf16, kind="ExternalOutput")

    # exp scale: logits_eff = S * 1/(2 sqrt C)  (see module docstring)
    exp_scale = 1.0 / (2.0 * math.sqrt(C))
    sig_scale = 1.0 / (S1 * S2)        # sigma(z2) from psum z2' = S1*S2*z2
    tanh_scale = 1.0 / (2.0 * S1 * S2)  # tanh(z2/2)

    with tile.TileContext(nc) as tc:
        with (
            tc.tile_pool(name="singles", bufs=1) as singles,
            tc.tile_pool(name="acts", bufs=2) as acts,
            tc.tile_pool(name="hts", bufs=3) as hts,
            tc.tile_pool(name="obuf", bufs=3) as obuf,
            tc.tile_pool(name="psum", bufs=4, space="PSUM") as psum,
        ):
            w1_sb = {}
            w2_sb = {}
            for kind in "qkv":
                w1_sb[kind] = singles.tile(
                    [P, KO1, HID], f8, tag=f"w1{kind}", name=f"w1{kind}"
                )
                w2_sb[kind] = singles.tile(
                    [P, KO2, HW], f8, tag=f"w2{kind}", name=f"w2{kind}"
                )

            warm_sb = singles.tile([P, C], f8, tag="warm", name="warm")
            nc.gpsimd.memset(warm_sb, 0.0)

            # Input DMAs in need-order: wq1 + eT(s0) first (first matmuls),
            # the rest behind. HWDGE (sync) and SWDGE (gpsimd) run in
            # parallel; split the critical first wave across both.
            nc.sync.dma_start(out=w1_sb["q"], in_=ws1["q"][:])
            eT0 = acts.tile([P, KO1, C], f8, tag="eT", name="eT")
            nc.gpsimd.dma_start(out=eT0[:, 6:, :], in_=xt[0][:, 6:, :])
            nc.sync.dma_start(out=eT0[:, :2, :], in_=xt[0][:, :2, :])
            nc.sync.dma_start(out=eT0[:, 2:4, :], in_=xt[0][:, 2:4, :])
            nc.sync.dma_start(out=eT0[:, 4:6, :], in_=xt[0][:, 4:6, :])
            # layer-1 weights before the layer-2 ones: they are needed
            # first (sample 0's L1k/L1v would stall behind w2 transfers)
            nc.sync.dma_start(out=w1_sb["k"], in_=ws1["k"][:])
            nc.sync.dma_start(out=w1_sb["v"], in_=ws1["v"][:])
            nc.sync.dma_start(out=w2_sb["q"], in_=ws2["q"][:])
            nc.sync.dma_start(out=w2_sb["k"], in_=ws2["k"][:])
            nc.sync.dma_start(out=w2_sb["v"], in_=ws2["v"][:])

            # ones = 2.0 so the Z reciprocal yields 0.5/Z directly
            ones_sb = singles.tile([P, 1], f8, tag="ones", name="ones")
            nc.vector.memset(ones_sb, 2.0)
            actw = singles.tile([P, 1], f32, tag="actw", name="actw")

            # PE clock warm-up (HAM un-throttles after ~3.4us of activity)
            warm_ps = psum.tile([P, 2, C], f32, tag="ps", name="ps")
            for _ in range(7):
                nc.tensor.matmul(
                    warm_ps[:, 0, :], warm_sb[:, :P], warm_sb, start=True, stop=True
                )
            # Preload the sigmoid table set (contains sigmoid+tanh) while
            # DMAs land.
            nc.scalar.activation(actw, warm_sb[:, :1], A.Sigmoid)

            def layer1(kind, eT, kporder):
                # layer 1: h^T[r, c], DoubleRow over k-chunk pairs, then
                # relu PSUM -> fp8 SBUF (both m-tiles in one DVE op). The
                # relu latency is hidden under the NEXT block's matmuls.
                ps1 = psum.tile([P, MT_H, C], f32, tag="ps", name="ps")
                for m in range(MT_H):
                    for kpi, kp in enumerate(kporder):
                        nc.tensor.matmul(
                            ps1[:, m, :],
                            w1_sb[kind][:, 2 * kp : 2 * kp + 2, m * P : (m + 1) * P],
                            eT[:, 2 * kp : 2 * kp + 2, :],
                            start=(kpi == 0),
                            stop=(kpi == KP1 - 1),
                            perf_mode=DR,
                        )
                # relu in two per-m halves: halves the DVE latency exposed
                # to the PE consumer and smooths the DVE queue
                hT = hts.tile([P, KO2, C], f8, tag="hT", name="hT")
                for m in range(MT_H):
                    nc.vector.tensor_scalar_max(hT[:, m, :], ps1[:, m, :], 0.0)
                return hT

            def layer2_qk_unit(kind, hT, dst, mp):
                # one m-pair of layer 2 transposed: z2^T[e, c]
                fn = A.Sigmoid if kind == "q" else A.Tanh
                sc = sig_scale if kind == "q" else tanh_scale
                ps2 = psum.tile([P, 2, C], f32, tag="ps", name="ps")
                for mi in range(2):
                    m = 2 * mp + mi
                    nc.tensor.matmul(
                        ps2[:, mi, :],
                        w2_sb[kind][:, 0:2, m * P : (m + 1) * P],
                        hT[:, 0:2, :],
                        start=True,
                        stop=True,
                        perf_mode=DR,
                    )
                nc.scalar.activation(
                    dst[:, 2 * mp : 2 * mp + 2, :], ps2[:], fn, scale=sc
                )

            def layer2_v_unit(hT, tv, m):
                # one m-tile of v natural: z2[d, e] = (h^T)^T @ w2
                ps2 = psum.tile([P, 2, C], f32, tag="ps", name="ps")
                for nh in range(NH):
                    nc.tensor.matmul(
                        ps2[:, nh, :],
                        hT[:, 0:2, m * P : (m + 1) * P],
                        w2_sb["v"][:, 0:2, nh * C : (nh + 1) * C],
                        start=True,
                        stop=True,
                        perf_mode=DR,
                    )
                nc.scalar.activation(tv[:, m, :], ps2[:], A.Tanh, scale=tanh_scale)

            def scores_unit(qT, kT, E, mp, split_exp=False):
                # one m-pair of S^T[d, c] = sum_e t_k[d,e] sigma_q[c,e];
                # returns the psum tile (its corner hosts a Z accumulator)
                ps = psum.tile([P, 2, C], f32, tag="ps", name="ps")
                for mi in range(2):
                    m = 2 * mp + mi
                    for kp in range(KPE):
                        nc.tensor.matmul(
                            ps[:, mi, :],
                            kT[:, 2 * kp : 2 * kp + 2, m * P : (m + 1) * P],
                            qT[:, 2 * kp : 2 * kp + 2, :],
                            start=(kp == 0),
                            stop=(kp == KPE - 1),
                            perf_mode=DR,
                        )
                if split_exp:
                    # final iteration: per-half exps let the epilogue's Z
                    # start one exp earlier
                    for mi in range(2):
                        nc.scalar.activation(
                            E[:, 2 * mp + mi, :], ps[:, mi, :], A.Exp,
                            scale=exp_scale,
                        )
                else:
                    nc.scalar.activation(
                        E[:, 2 * mp : 2 * mp + 2, :], ps[:, :, :], A.Exp,
                        scale=exp_scale,
                    )
                return ps

            def z_phase(E, scps):
                # softmax denominator rz[c] = 0.5/Z[c] (ones = 2.0). The 4
                # accumulators live in the corner columns of the PREVIOUS
                # iteration's two scores tiles — one per PSUM bank (groups
                # sharing a bank would serialize), and the tile-WAR against
                # the exp reads is exactly the true dependency. This frees
                # two banks, buying the main pool its 4th buffer.
                rz = obuf.tile([P, MT_C], f32, tag="rz", name="rz")
                for m in range(MT_C):
                    pz = scps[m // 2][:, m % 2, C - 1 : C]
                    for k in range(MT_C):
                        nc.tensor.matmul(
                            pz,
                            E[:, k, m * P : (m + 1) * P],
                            ones_sb,
                            start=(k == 0),
                            stop=(k == MT_C - 1),
                        )
                    nc.vector.reciprocal(rz[:, m : m + 1], pz)
                return rz

            def o_unit(s, E, tv, rz, m, split):
                # one m-tile of o[c, e] = (E^T @ t_v) * rz[c] + 0.5
                ob = obuf.tile([P, HW], bf16, tag="ob", name="ob")
                ps = psum.tile([P, 2, C], f32, tag="ps", name="ps")
                out_r = out[s].rearrange("(mo p) e -> p mo e", p=P)
                for kp in range(KPC):
                    for nh in range(NH):
                        nc.tensor.matmul(
                            ps[:, nh, :],
                            E[:, 2 * kp : 2 * kp + 2, m * P : (m + 1) * P],
                            tv[:, 2 * kp : 2 * kp + 2, nh * C : (nh + 1) * C],
                            start=(kp == 0),
                            stop=(kp == KPC - 1),
                            perf_mode=DR,
                        )
                if not split:
                    nc.vector.tensor_scalar(
                        ob[:], ps[:], rz[:, m : m + 1], 0.5, ALU.mult, ALU.add
                    )
                    eng = (nc.sync, nc.scalar)[m % 2]
                    eng.dma_start(out=out_r[:, m, :], in_=ob[:])
                else:
                    # final sample: the epilogue has no matmuls to hide
                    # behind, so the scale+store drain chain is the tail.
                    # Split into halves and alternate the scale between the
                    # DVE and the (now idle) ACT engine — Copy activation
                    # computes ps*rz + 0.5 directly.
                    for nh in range(NH):
                        dst = ob[:, nh * C : (nh + 1) * C]
                        src = ps[:, nh, :]
                        if (m * NH + nh) % 2 == 0:
                            nc.vector.tensor_scalar(
                                dst, src, rz[:, m : m + 1], 0.5, ALU.mult, ALU.add
                            )
                        else:
                            # ACT is idle in the epilogue; Copy computes
                            # ps*rz + 0.5 directly, halving the drain chain
                            nc.scalar.activation(
                                dst, src, A.Copy, bias=0.5, scale=rz[:, m : m + 1]
                            )
                        eng = (nc.sync, nc.scalar)[(m * NH + nh) % 2]
                        eng.dma_start(
                            out=out_r[:, m, nh * C : (nh + 1) * C],
                            in_=dst,
                        )

            # Software pipeline. Per iteration, PE-only units (layer-1
            # blocks, Z, and the PREVIOUS sample's o-tiles) are woven
            # between the ACT-fed layer-2 / scores units so the scalar
            # engine (the second-busiest) never backs the PSUM pool up
            # into the PE. exp/reciprocal of sample s complete during
            # iteration s+1 long before their consumers.
            prev = None
            eT_next = None
            for s in range(PER):
                eT = eT0 if s == 0 else eT_next

                qT = acts.tile([P, MT_E, C], f8, tag="qT", name="qT")
                kT = acts.tile([P, MT_E, C], f8, tag="kT", name="kT")
                tv = acts.tile([P, MT_C, HW], f8, tag="tv", name="tv")

                # sample 0's eT pairs arrive sync(0,1),(2,3),(4,5) +
                # gpsimd(6,7); accumulate in arrival order (order is free)
                kporder = (0, 1, 3, 2) if s == 0 else tuple(range(KP1))
                hq = layer1("q", eT, kporder)
                hk = layer1("k", eT, kporder)
                if prev is not None:
                    # Z early: its reciprocals clear the DVE queue before
                    # the relu halves and o-scales pile up
                    ps_, E_, tv_, scps_ = prev
                    rz_ = z_phase(E_, scps_)
                layer2_qk_unit("q", hq, qT, 0)
                layer2_qk_unit("q", hq, qT, 1)
                hv = layer1("v", eT, kporder)
                if s + 1 < PER:
                    # prefetch next sample's eT on the sync HWDGE queue;
                    # issued after this sample's layer-1 so the transfer
                    # never contends with the startup input DMAs
                    eT_next = acts.tile([P, KO1, C], f8, tag="eT", name="eT")
                    nc.sync.dma_start(out=eT_next, in_=xt[s + 1])
                layer2_qk_unit("q", hq, qT, 2)
                layer2_qk_unit("q", hq, qT, 3)
                # prefetch the exp table right after the LAST sigmoid user:
                # tanh lives in the exp table set too, so the 1.28us load
                # hides under the k/v-tanh window instead of delaying the
                # exps (and the epilogue) by a full table-load
                nc.scalar.activation(actw, qT[:, MT_E - 1, :1], A.Exp)
                layer2_qk_unit("k", hk, kT, 0)
                layer2_qk_unit("k", hk, kT, 1)
                if prev is not None:
                    o_unit(ps_, E_, tv_, rz_, 0, False)
                layer2_qk_unit("k", hk, kT, 2)
                layer2_qk_unit("k", hk, kT, 3)
                if prev is not None:
                    o_unit(ps_, E_, tv_, rz_, 1, False)
                    o_unit(ps_, E_, tv_, rz_, 2, False)
                layer2_v_unit(hv, tv, 0)
                layer2_v_unit(hv, tv, 1)
                layer2_v_unit(hv, tv, 2)
                layer2_v_unit(hv, tv, 3)
                if prev is not None:
                    o_unit(ps_, E_, tv_, rz_, 3, False)

                E = acts.tile([P, MT_C, C], f8, tag="E", name="E")
                last = s == PER - 1
                scp0 = scores_unit(qT, kT, E, 0, split_exp=last)
                scp1 = scores_unit(qT, kT, E, 1, split_exp=last)
                if not last:
                    # pull the sigmoid-table reload into the next window
                    nc.scalar.activation(actw, E[:, MT_C - 1, :1], A.Sigmoid)
                prev = (s, E, tv, (scp0, scp1))

            # epilogue: final sample's Z + output drain
            s_, E_, tv_, scps_ = prev
            rz_ = z_phase(E_, scps_)
            for m in range(MT_C):
                o_unit(s_, E_, tv_, rz_, m, m == MT_C - 1)

    nc.finalize()
    return nc


def _get_nc():
    if "nc" not in _STATE:
        _STATE["nc"] = _build_nc()
    return _STATE["nc"]


def _quant_ef(a):
    """fp8 quantization with error feedback along axis 0 (contraction axis):
    carry the rounding residual so each column's sum is preserved."""
    out = np.empty(a.shape, dtype=FP8)
    c = np.zeros(a.shape[1:], dtype=np.float32)
    for h in range(a.shape[0]):
        u = a[h] + c
        q = u.astype(FP8)
        c = u - q.astype(np.float32)
        out[h] = q
    return out


def kernel(**inputs):
    x = np.asarray(inputs["x"])

    # host-side reformat to SBUF-native layouts (+ fp8 cast):
    #   x:  [N, C, H, W] -> e^T [N, HW, C] -> [N, P, KO1, C]
    #   w1: [HW, HID] * 16 -> [P, KO1, HID]
    #   w2: [HID, HW] * 16 -> [P, KO2, HW]  (wv2 with error feedback)
    xt = np.ascontiguousarray(
        x.reshape(N, C, HW)
        .transpose(0, 2, 1)
        .reshape(N, KO1, P, C)
        .transpose(0, 2, 1, 3)
    ).astype(FP8)
    w = {}
    for name in ("wq1", "wk1", "wv1"):
        a = (np.asarray(inputs[name]) * S1).astype(FP8)
        w[name] = np.ascontiguousarray(a.reshape(KO1, P, HID).transpose(1, 0, 2))
    for name in ("wq2", "wk2", "wv2"):
        a = np.asarray(inputs[name]).astype(np.float32) * S2
        a8 = _quant_ef(a) if name == "wv2" else a.astype(FP8)
        w[name] = np.ascontiguousarray(a8.reshape(KO2, P, HW).transpose(1, 0, 2))

    nc = _get_nc()

    in_maps = []
    for c in range(NCORES):
        m = {"xt": np.ascontiguousarray(xt[c * PER : (c + 1) * PER])}
        for kind in "qkv":
            m[f"w{kind}1"] = w[f"w{kind}1"]
            m[f"w{kind}2"] = w[f"w{kind}2"]
        in_maps.append(m)

    from concourse.bass_utils import run_bass_kernel_spmd

    res = run_bass_kernel_spmd(
        nc,
        in_maps,
        core_ids=list(range(NCORES)),
        trace=_STATE.get("trace", False),
        **_STATE.get("run_kwargs", {}),
    )
    _STATE["last_result"] = res

    o = np.concatenate([r["o"] for r in res.results], axis=0)
    return o.reshape(N, C, H, W).astype(np.float32)


# revision 45
# speedup vs baseline: 1.0559x; 1.0521x over previous
"""Trainium2 Bass kernel for per-sample channel attention (fp8 DoubleRow).

Reference computation (per sample n of 32):
    e  = x[n].reshape(C, HW)                      # C=512, HW=1024
    q  = sigmoid(relu(e @ wq1) @ wq2)             # [C, HW]
    k  = sigmoid(relu(e @ wk1) @ wk2)             # [C, HW]
    v  = sigmoid(relu(e @ wv1) @ wv2)             # [C, HW]
    s  = q @ k.T / sqrt(C)                        # [C, C]
    o  = softmax(s, axis=-1) @ v                  # [C, HW]

Strategy: data-parallel over batch N across 8 cores (4 samples each),
weights replicated. All big matmuls are fp8 (e4m3) with
perf_mode=DoubleRow; measured back-to-back DR matmuls stream at 216ns
per [128k x 2pair] x [128 x 512] — the FULL 2x-over-bf16 fp8 rate
(~69us/core of pure matmul for the 320 DR matmuls). PSUM accumulates
fp32. Measured HW exec: ~103-107us/core (vs 164us bf16 baseline);
occasional chip-level clock-throttle outliers to ~125us+.

Numerics (absmax rel err ~6.8e-3 vs the 2e-2 gate):
  - k,v are stored as t = tanh(z/2) in (-1,1) (sigmoid = (1+t)/2), q as
    sigma(z) in (0,1). Identities absorb all affine corrections:
      softmax_d(q·sigma_k) == softmax_d(S * 1/(2 sqrt C)),
        S[d,c] = sum_e t_k[d,e]·sigma_q[c,e]   (the q-rowsum term is
        constant per softmax row and cancels)
      P @ v = (P @ t_v + 1)/2                   (softmax rows sum to 1)
    Only two ACT table switches per sample; both prefetched into slack
    via dummy activations.
  - wv2 is quantized with error feedback along the contraction axis so
    each column's sum survives quantization; this kills the dominant
    error term (relu(h) has positive mean, so plain wv2 rounding noise
    is a common mode the softmax averaging cannot cancel). wq2/wk2
    common modes cancel through the softmax and need no treatment.
  - w1,w2 are pre-scaled by 16 on host (fp8 subnormal avoidance); the
    1/256 folds into the ACT scale. Z uses a ones=2.0 vector so the
    reciprocal directly yields 0.5/Z, folding the (1+t)/2 un-mapping
    into the existing per-partition output scale. Output DMA in bf16,
    upcast on host.

Scheduling (the Tile list-scheduler plans via CoreSim):
  - _patch_sched_cost_model doubles the sim's PE cost constants so the
    scheduler sees the TRUE 1.0 cycle/row DoubleRow rate instead of its
    0.5 model; this alone was worth ~12us (without it the sim's 2x-fast
    PE leapfrogs phases around PSUM-slot waits and stalls real HW).
  - software pipeline: Z + the 4 output tiles of sample s-1 run as
    PE-only filler woven between sample s's ACT-fed layer-2 units, so
    exp/reciprocal latency never touches the PE critical path.
  - the 4 softmax-denominator accumulators live in corner columns of
    the previous iteration's two scores PSUM tiles (one per bank; the
    tile-WAR vs the exp reads is the true dependency anyway). That
    frees 2 banks, buying the main [P,2,C] PSUM pool its 4th buffer --
    every pool-rotation wait then lands 4 allocations back, on ACT/DVE
    work that has already drained.
  - relus are split into per-m halves (halves the DVE latency exposed
    to layer-2), layer-1 weight DMAs precede layer-2's, eT prefetch
    rides the sync HWDGE queue mid-iteration, and the epilogue splits
    its tail scale+store into halves alternating DVE/ACT.

Layouts (all SBUF-native, partition-major, pre-swizzled on host):
  eT   [P, KO1, C]  e^T chunks       qT [P, MT_E, C]  sigma_q^T
  kT   [P, MT_E, C] t_k^T            tv [P, MT_C, HW] t_v (natural)
  E    [P, MT_C, C] exp(scores^T)    softmax axis on partitions
DoubleRow contracts k-chunk PAIRS: lhsT [P, 2, M], rhs [P, 2, N=512].
"""

import math

import numpy as np
import ml_dtypes

N, C, H, W, R = 32, 512, 32, 32, 4
HW = H * W            # 1024
HID = HW // R         # 256
NCORES = 8
PER = N // NCORES     # samples per core
P = 128               # partitions

KO1 = HW // P         # 8  k-chunks for layer1
KP1 = KO1 // 2        # 4  DoubleRow pairs for layer1
KO2 = HID // P        # 2  k-chunks for layer2 (one DoubleRow pair)
MT_H = HID // P       # 2  m-tiles of h^T
MT_E = HW // P        # 8  m-tiles of q^T/k^T
KPE = MT_E // 2       # 4  DoubleRow pairs for scores contraction
MT_C = C // P         # 4  m-tiles over C
KPC = MT_C // 2       # 2  DoubleRow pairs for o contraction
NH = HW // C          # 2  halves of HW free dim (512 each)

S1 = 16.0             # host pre-scale on w1 (fp8 range use)
S2 = 16.0             # host pre-scale on w2

_STATE = {}

FP8 = ml_dtypes.float8_e4m3


def _patch_sched_cost_model():
    """Make the Tile scheduler's CoreSim cost model match measured HW.

    CoreSim costs fp8 DoubleRow matmuls at 0.5 cycles/row (107ns @ N=512)
    but the hardware streams them at 1.0 cycle/row (measured 216ns
    back-to-back). The list scheduler therefore simulates a PE twice as
    fast as reality and leapfrogs whole phases around PSUM-slot waits,
    producing avoidable PE stalls on hardware. Doubling the PE cost
    constants (loaded lazily by the Rust cost model from this module)
    restores the true PE:ACT:DVE timing ratios. Sim-only: NEFF codegen
    never reads these.
    """
    import concourse.hw_specs as hs

    spec = hs.TRN2Spec
    if getattr(spec, "_dr_cost_patched", False):
        return
    spec._dr_cost_patched = True
    spec.PE_CYCLE *= 2.0
    spec.PE_CYCLE_PSTATE_MID *= 2.0
    spec.PE_CYCLE_PSTATE_LOW *= 2.0
    spec.PE_SBUF_ACCESS_LATENCY_NS *= 2.0


def _build_nc():
    import concourse.bass as bass  # noqa: F401
    import concourse.mybir as mybir
    import concourse.tile as tile
    from concourse import bacc

    _patch_sched_cost_model()

    f8 = mybir.dt.float8e4
    f32 = mybir.dt.float32
    A = mybir.ActivationFunctionType
    DR = mybir.MatmulPerfMode.DoubleRow
    ALU = mybir.AluOpType

    nc = bacc.Bacc("TRN2")

    xt = nc.dram_tensor("xt", [PER, P, KO1, C], f8, kind="ExternalInput")
    ws1 = {
        kind: nc.dram_tensor(f"w{kind}1", [P, KO1, HID], f8, kind="ExternalInput")
        for kind in "qkv"
    }
    ws2 = {
        kind: nc.dram_tensor(f"w{kind}2", [P, KO2, HW], f8, kind="ExternalInput")
        for kind in "qkv"
    }
    bf16 = mybir.dt.bfloat16
    # output in bf16 (upcast on host): halves output DMA traffic; adds
    # <=2e-3 abs error on ~0.9-scale outputs, well within the error budget
    out = nc.dram_tensor("o", [PER, C, HW], bf16, kind="ExternalOutput")

    # exp scale: logits_eff = S * 1/(2 sqrt C)  (see module docstring)
    exp_scale = 1.0 / (2.0 * math.sqrt(C))
    sig_scale = 1.0 / (S1 * S2)        # sigma(z2) from psum z2' = S1*S2*z2
    tanh_scale = 1.0 / (2.0 * S1 * S2)  # tanh(z2/2)

    with tile.TileContext(nc) as tc:
        with (
            tc.tile_pool(name="singles", bufs=1) as singles,
            tc.tile_pool(name="acts", bufs=2) as acts,
            tc.tile_pool(name="hts", bufs=3) as hts,
            tc.tile_pool(name="obuf", bufs=3) as obuf,
            tc.tile_pool(name="psum", bufs=4, space="PSUM") as psum,
        ):
            w1_sb = {}
            w2_sb = {}
            for kind in "qkv":
                w1_sb[kind] = singles.tile(
                    [P, KO1, HID], f8, tag=f"w1{kind}", name=f"w1{kind}"
                )
                w2_sb[kind] = singles.tile(
                    [P, KO2, HW], f8, tag=f"w2{kind}", name=f"w2{kind}"
                )

            warm_sb = singles.tile([P, C], f8, tag="warm", name="warm")
            nc.gpsimd.memset(warm_sb, 0.0)

            # Input DMAs in need-order: wq1 + eT(s0) first (first matmuls),
            # the rest behind. HWDGE (sync) and SWDGE (gpsimd) run in
            # parallel; split the critical first wave across both.
            nc.sync.dma_start(out=w1_sb["q"], in_=ws1["q"][:])
            eT0 = acts.tile([P, KO1, C], f8, tag="eT", name="eT")
            nc.gpsimd.dma_start(out=eT0[:, 6:, :], in_=xt[0][:, 6:, :])
            nc.sync.dma_start(out=eT0[:, :2, :], in_=xt[0][:, :2, :])
            nc.sync.dma_start(out=eT0[:, 2:4, :], in_=xt[0][:, 2:4, :])
            nc.sync.dma_start(out=eT0[:, 4:6, :], in_=xt[0][:, 4:6, :])
            # layer-1 weights before the layer-2 ones: they are needed
            # first (sample 0's L1k/L1v would stall behind w2 transfers)
            nc.sync.dma_start(out=w1_sb["k"], in_=ws1["k"][:])
            nc.sync.dma_start(out=w1_sb["v"], in_=ws1["v"][:])
            nc.sync.dma_start(out=w2_sb["q"], in_=ws2["q"][:])
            nc.sync.dma_start(out=w2_sb["k"], in_=ws2["k"][:])
            nc.sync.dma_start(out=w2_sb["v"], in_=ws2["v"][:])

            # ones = 2.0 so the Z reciprocal yields 0.5/Z directly
            ones_sb = singles.tile([P, 1], f8, tag="ones", name="ones")
            nc.vector.memset(ones_sb, 2.0)
            actw = singles.tile([P, 1], f32, tag="actw", name="actw")

            # PE clock warm-up (HAM un-throttles after ~3.4us of activity)
            warm_ps = psum.tile([P, 2, C], f32, tag="ps", name="ps")
            for _ in range(7):
                nc.tensor.matmul(
                    warm_ps[:, 0, :], warm_sb[:, :P], warm_sb, start=True, stop=True
                )
            # Preload the sigmoid table set (contains sigmoid+tanh) while
            # DMAs land.
            nc.scalar.activation(actw, warm_sb[:, :1], A.Sigmoid)

            def layer1(kind, eT, kporder):
                # layer 1: h^T[r, c], DoubleRow over k-chunk pairs, then
                # relu PSUM -> fp8 SBUF (both m-tiles in one DVE op). The
                # relu latency is hidden under the NEXT block's matmuls.
                ps1 = psum.tile([P, MT_H, C], f32, tag="ps", name="ps")
                for m in range(MT_H):
                    for kpi, kp in enumerate(kporder):
                        nc.tensor.matmul(
                            ps1[:, m, :],
                            w1_sb[kind][:, 2 * kp : 2 * kp + 2, m * P : (m + 1) * P],
                            eT[:, 2 * kp : 2 * kp + 2, :],
                            start=(kpi == 0),
                            stop=(kpi == KP1 - 1),
                            perf_mode=DR,
                        )
                # relu in two per-m halves: halves the DVE latency exposed
                # to the PE consumer and smooths the DVE queue
                hT = hts.tile([P, KO2, C], f8, tag="hT", name="hT")
                for m in range(MT_H):
                    nc.vector.tensor_scalar_max(hT[:, m, :], ps1[:, m, :], 0.0)
                return hT

            def layer2_qk_unit(kind, hT, dst, mp):
                # one m-pair of layer 2 transposed: z2^T[e, c]
                fn = A.Sigmoid if kind == "q" else A.Tanh
                sc = sig_scale if kind == "q" else tanh_scale
                ps2 = psum.tile([P, 2, C], f32, tag="ps", name="ps")
                for mi in range(2):
                    m = 2 * mp + mi
                    nc.tensor.matmul(
                        ps2[:, mi, :],
                        w2_sb[kind][:, 0:2, m * P : (m + 1) * P],
                        hT[:, 0:2, :],
                        start=True,
                        stop=True,
                        perf_mode=DR,
                    )
                nc.scalar.activation(
                    dst[:, 2 * mp : 2 * mp + 2, :], ps2[:], fn, scale=sc
                )

            def layer2_v_unit(hT, tv, m):
                # one m-tile of v natural: z2[d, e] = (h^T)^T @ w2
                ps2 = psum.tile([P, 2, C], f32, tag="ps", name="ps")
                for nh in range(NH):
                    nc.tensor.matmul(
                        ps2[:, nh, :],
                        hT[:, 0:2, m * P : (m + 1) * P],
                        w2_sb["v"][:, 0:2, nh * C : (nh + 1) * C],
                        start=True,
                        stop=True,
                        perf_mode=DR,
                    )
                nc.scalar.activation(tv[:, m, :], ps2[:], A.Tanh, scale=tanh_scale)

            def scores_unit(qT, kT, E, mp, split_exp=False):
                # one m-pair of S^T[d, c] = sum_e t_k[d,e] sigma_q[c,e];
                # returns the psum tile (its corner hosts a Z accumulator)
                ps = psum.tile([P, 2, C], f32, tag="ps", name="ps")
                for mi in range(2):
                    m = 2 * mp + mi
                    for kp in range(KPE):
                        nc.tensor.matmul(
                            ps[:, mi, :],
                            kT[:, 2 * kp : 2 * kp + 2, m * P : (m + 1) * P],
                            qT[:, 2 * kp : 2 * kp + 2, :],
                            start=(kp == 0),
                            stop=(kp == KPE - 1),
                            perf_mode=DR,
                        )
                if split_exp:
                    # final iteration: per-half exps let the epilogue's Z
                    # start one exp earlier
                    for mi in range(2):
                        nc.scalar.activation(
                            E[:, 2 * mp + mi, :], ps[:, mi, :], A.Exp,
                            scale=exp_scale,
                        )
                else:
                    nc.scalar.activation(
                        E[:, 2 * mp : 2 * mp + 2, :], ps[:, :, :], A.Exp,
                        scale=exp_scale,
                    )
                return ps

            def z_phase(E, scps):
                # softmax denominator rz[c] = 0.5/Z[c] (ones = 2.0). The 4
                # accumulators live in the corner columns of the PREVIOUS
                # iteration's two scores tiles — one per PSUM bank (groups
                # sharing a bank would serialize), and the tile-WAR against
                # the exp reads is exactly the true dependency. This frees
                # two banks, buying the main pool its 4th buffer.
                rz = obuf.tile([P, MT_C], f32, tag="rz", name="rz")
                for m in range(MT_C):
                    pz = scps[m // 2][:, m % 2, C - 1 : C]
                    for k in range(MT_C):
                        nc.tensor.matmul(
                            pz,
                            E[:, k, m * P : (m + 1) * P],
                            ones_sb,
                            start=(k == 0),
                            stop=(k == MT_C - 1),
                        )
                    nc.vector.reciprocal(rz[:, m : m + 1], pz)
                return rz

            def o_unit(s, E, tv, rz, m, split):
                # one m-tile of o[c, e] = (E^T @ t_v) * rz[c] + 0.5
                ob = obuf.tile([P, HW], bf16, tag="ob", name="ob")
                ps = psum.tile([P, 2, C], f32, tag="ps", name="ps")
                out_r = out[s].rearrange("(mo p) e -> p mo e", p=P)
                for kp in range(KPC):
                    for nh in range(NH):
                        nc.tensor.matmul(
                            ps[:, nh, :],
                            E[:, 2 * kp : 2 * kp + 2, m * P : (m + 1) * P],
                            tv[:, 2 * kp : 2 * kp + 2, nh * C : (nh + 1) * C],
                            start=(kp == 0),
                            stop=(kp == KPC - 1),
                            perf_mode=DR,
                        )
                if not split:
                    nc.vector.tensor_scalar(
                        ob[:], ps[:], rz[:, m : m + 1], 0.5, ALU.mult, ALU.add
                    )
                    eng = (nc.sync, nc.scalar)[m % 2]
                    eng.dma_start(out=out_r[:, m, :], in_=ob[:])
                else:
                    # final sample: the epilogue has no matmuls to hide
                    # behind, so the scale+store drain chain is the tail.
                    # Split into halves and alternate the scale between the
                    # DVE and the (now idle) ACT engine — Copy activation
                    # computes ps*rz + 0.5 directly.
                    for nh in range(NH):
                        dst = ob[:, nh * C : (nh + 1) * C]
                        src = ps[:, nh, :]
                        if (m * NH + nh) % 2 == 0:
                            nc.vector.tensor_scalar(
                                dst, src, rz[:, m : m + 1], 0.5, ALU.mult, ALU.add
                            )
                        else:
                            # ACT is idle in the epilogue; Copy computes
                            # ps*rz + 0.5 directly, halving the drain chain
                            nc.scalar.activation(
                                dst, src, A.Copy, bias=0.5, scale=rz[:, m : m + 1]
                            )
                        eng = (nc.sync, nc.scalar)[(m * NH + nh) % 2]
                        eng.dma_start(
                            out=out_r[:, m, nh * C : (nh + 1) * C],
                            in_=dst,
                        )

            # Software pipeline. Per iteration, PE-only units (layer-1
            # blocks, Z, and the PREVIOUS sample's o-tiles) are woven
            # between the ACT-fed layer-2 / scores units so the scalar
            # engine (the second-busiest) never backs the PSUM pool up
            # into the PE. exp/reciprocal of sample s complete during
            # iteration s+1 long before their consumers.
            prev = None
            eT_next = None
            for s in range(PER):
                eT = eT0 if s == 0 else eT_next

                qT = acts.tile([P, MT_E, C], f8, tag="qT", name="qT")
                kT = acts.tile([P, MT_E, C], f8, tag="kT", name="kT")
                tv = acts.tile([P, MT_C, HW], f8, tag="tv", name="tv")

                # sample 0's eT pairs arrive sync(0,1),(2,3),(4,5) +
                # gpsimd(6,7); accumulate in arrival order (order is free)
                kporder = (0, 1, 3, 2) if s == 0 else tuple(range(KP1))
                hq = layer1("q", eT, kporder)
                hk = layer1("k", eT, kporder)
                if prev is not None:
                    # Z early: its reciprocals clear the DVE queue before
                    # the relu halves and o-scales pile up
                    ps_, E_, tv_, scps_ = prev
                    rz_ = z_phase(E_, scps_)
                layer2_qk_unit("q", hq, qT, 0)
                layer2_qk_unit("q", hq, qT, 1)
                hv = layer1("v", eT, kporder)
                if s + 1 < PER:
                    # prefetch next sample's eT on the sync HWDGE queue;
                    # issued after this sample's layer-1 so the transfer
                    # never contends with the startup input DMAs
                    eT_next = acts.tile([P, KO1, C], f8, tag="eT", name="eT")
                    nc.sync.dma_start(out=eT_next, in_=xt[s + 1])
                layer2_qk_unit("q", hq, qT, 2)
                layer2_qk_unit("q", hq, qT, 3)
                if s == PER - 1:
                    # final iteration: prefetch the exp table right after the
                    # LAST sigmoid user — tanh lives in the exp table set
                    # too, so the 1.28us load hides under the k/v-tanh
                    # window and the epilogue's exps finish a full
                    # table-load earlier. (Mid-iterations keep the load in
                    # the scores window: hoisting there delays the k-tanh
                    # chain instead.)
                    nc.scalar.activation(actw, qT[:, MT_E - 1, :1], A.Exp)
                layer2_qk_unit("k", hk, kT, 0)
                layer2_qk_unit("k", hk, kT, 1)
                if prev is not None:
                    o_unit(ps_, E_, tv_, rz_, 0, False)
                layer2_qk_unit("k", hk, kT, 2)
                layer2_qk_unit("k", hk, kT, 3)
                if prev is not None:
                    o_unit(ps_, E_, tv_, rz_, 1, False)
                    o_unit(ps_, E_, tv_, rz_, 2, False)
                layer2_v_unit(hv, tv, 0)
                layer2_v_unit(hv, tv, 1)
                layer2_v_unit(hv, tv, 2)
                layer2_v_unit(hv, tv, 3)
                if prev is not None:
                    o_unit(ps_, E_, tv_, rz_, 3, False)
                if s < PER - 1:
                    # prefetch the exp table (depends on the last tanh)
                    nc.scalar.activation(actw, tv[:, MT_C - 1, HW - 1 :], A.Exp)

                E = acts.tile([P, MT_C, C], f8, tag="E", name="E")
                last = s == PER - 1
                scp0 = scores_unit(qT, kT, E, 0, split_exp=last)
                scp1 = scores_unit(qT, kT, E, 1, split_exp=last)
                if not last:
                    # pull the sigmoid-table reload into the next window
                    nc.scalar.activation(actw, E[:, MT_C - 1, :1], A.Sigmoid)
                prev = (s, E, tv, (scp0, scp1))

            # epilogue: final sample's Z + output drain
            s_, E_, tv_, scps_ = prev
            rz_ = z_phase(E_, scps_)
            for m in range(MT_C):
                o_unit(s_, E_, tv_, rz_, m, m == MT_C - 1)

    nc.finalize()
    return nc


def _get_nc():
    if "nc" not in _STATE:
        _STATE["nc"] = _build_nc()
    return _STATE["nc"]


def _quant_ef(a):
    """fp8 quantization with error feedback along axis 0 (contraction axis):
    carry the rounding residual so each column's sum is preserved."""
    out = np.empty(a.shape, dtype=FP8)
    c = np.zeros(a.shape[1:], dtype=np.float32)
    for h in range(a.shape[0]):
        u = a[h] + c
        q = u.astype(FP8)
        c = u - q.astype(np.float32)
        out[h] = q
    return out


def kernel(**inputs):
    x = np.asarray(inputs["x"])

    # host-side reformat to SBUF-native layouts (+ fp8 cast):
    #   x:  [N, C, H, W] -> e^T [N, HW, C] -> [N, P, KO1, C]
    #   w1: [HW, HID] * 16 -> [P, KO1, HID]
    #   w2: [HID, HW] * 16 -> [P, KO2, HW]  (wv2 with error feedback)
    xt = np.ascontiguousarray(
        x.reshape(N, C, HW)
        .transpose(0, 2, 1)
        .reshape(N, KO1, P, C)
        .transpose(0, 2, 1, 3)
    ).astype(FP8)
    w = {}
    for name in ("wq1", "wk1", "wv1"):
        a = (np.asarray(inputs[name]) * S1).astype(FP8)
        w[name] = np.ascontiguousarray(a.reshape(KO1, P, HID).transpose(1, 0, 2))
    for name in ("wq2", "wk2", "wv2"):
        a = np.asarray(inputs[name]).astype(np.float32) * S2
        a8 = _quant_ef(a) if name == "wv2" else a.astype(FP8)
        w[name] = np.ascontiguousarray(a8.reshape(KO2, P, HW).transpose(1, 0, 2))

    nc = _get_nc()

    in_maps = []
    for c in range(NCORES):
        m = {"xt": np.ascontiguousarray(xt[c * PER : (c + 1) * PER])}
        for kind in "qkv":
            m[f"w{kind}1"] = w[f"w{kind}1"]
            m[f"w{kind}2"] = w[f"w{kind}2"]
        in_maps.append(m)

    from concourse.bass_utils import run_bass_kernel_spmd

    res = run_bass_kernel_spmd(
        nc,
        in_maps,
        core_ids=list(range(NCORES)),
        trace=_STATE.get("trace", False),
        **_STATE.get("run_kwargs", {}),
    )
    _STATE["last_result"] = res

    o = np.concatenate([r["o"] for r in res.results], axis=0)
    return o.reshape(N, C, H, W).astype(np.float32)
